# revision 19
# baseline (speedup 1.0000x reference)
"""Trainium2 Bass kernel for nn_ModelNew_3556232921881 (dense_mlp).

Computes, for x[4096,4096], weight[4096,4096], bias[4096]:
    y = x @ weight.T + bias
    per-256-column-block mean subtraction (divided by out_features)
    tanh-approx GELU with clamped tanh

Sharding: 2 batch shards x 4 out-feature shards across 8 NeuronCores.
Per core: M=2048, N=1024, K=4096 GEMM (bf16 full-rate matmul, fp32 PSUM
accumulate) with a fused epilogue (bias add -> block reduce ->
Gelu_apprx_tanh with the negated block mean as per-partition activation
bias).

bf16 operands (vs the earlier fp32r version) because the fp32r
LDWEIGHTS is a 2-cycle-per-column load (~227ns) that cannot hide under
the 213ns N=512 matmul, capping the steady-state MM issue rate at
~272ns. bf16 LDWEIGHTS (~53-107ns, FWL-eligible) hides completely ->
~213ns/MM, and the W/x DMA footprint halves (W shard 8MB SBUF-resident,
1MB x half-tiles). Accumulation is fp32 in PSUM; measured rel err vs
the fp32 reference is ~2e-3 (gate 2e-2).

Host side pre-rounds x/weight to bf16 (RNE) and swizzles them into the
exact SBUF layouts so the device does zero transposes or dtype
conversions. The first 4 m-tiles run k-synchronously with the W DMA
stream so the PE never waits for the W preload.
"""

import numpy as np
from contextlib import ExitStack

B, IN_F, OUT_F = 4096, 4096, 4096
P_B, P_O = 2, 4          # batch shards x out-feature shards
MB = B // P_B            # 2048 rows per core
NB = OUT_F // P_O        # 1024 out cols per core
K = IN_F
P = 128
M_TILES = MB // P        # 16
KO = K // P              # 32
N_TILES = NB // 512      # 2
N_CORES = 8
WARM_G = 4               # m-tiles processed k-synchronously with W stream
WARMUP_MMS = 12          # throwaway MMs to lift the HAM clock-gate early

_STATE: dict = {}


def _to_bf16(a: np.ndarray) -> np.ndarray:
    """fp32 -> bf16 with round-to-nearest-even."""
    import ml_dtypes

    return np.ascontiguousarray(a, dtype=np.float32).astype(ml_dtypes.bfloat16)


def _build_bass(loop_reps=None, warm_group=WARM_G):
    import concourse.bass as bass  # noqa: F401
    import concourse.tile as tile
    from concourse import bacc, mybir

    f32 = mybir.dt.float32
    bf16 = mybir.dt.bfloat16
    AF = mybir.ActivationFunctionType

    nc = bacc.Bacc("TRN2", target_bir_lowering=False, debug=False)

    # element [p, m, ko, b] = xr[m*128+b, ko*128+p]  (per-core batch shard)
    xs_d = nc.dram_tensor("xs", [P, M_TILES, KO, P], bf16, kind="ExternalInput")
    # element [p, ko, n] = w[n, ko*128+p]            (per-core outf shard)
    ws_d = nc.dram_tensor("ws", [P, KO, NB], bf16, kind="ExternalInput")
    bb_d = nc.dram_tensor("bb", [P, NB], f32, kind="ExternalInput")
    out_d = nc.dram_tensor("out", [MB, NB], f32, kind="ExternalOutput")

    with tile.TileContext(nc) as tc:
        with ExitStack() as ctx:
            wpool = ctx.enter_context(tc.tile_pool(name="w", bufs=1))
            xpool = ctx.enter_context(tc.tile_pool(name="x", bufs=max(warm_group, 2)))
            ypool = ctx.enter_context(tc.tile_pool(name="y", bufs=2))
            gpool = ctx.enter_context(tc.tile_pool(name="g", bufs=2))
            spool = ctx.enter_context(tc.tile_pool(name="s", bufs=3))
            psum = ctx.enter_context(tc.tile_pool(name="ps", bufs=8, space="PSUM"))

            def epilogue(m, n, ps_t, bb_t, split=False):
                """Bias add -> per-256-block sum -> Gelu(bias=-mean).

                split=True pipelines the two 256-col blocks as independent
                vector->scalar->DMA chains; only worth it for the very last
                tile, where the chain is on the kernel's critical path.
                """
                nsl = slice(n * 512, (n + 1) * 512)
                if not split:
                    y1 = ypool.tile([P, 512], f32, name="y1")
                    nc.vector.tensor_add(y1[:], ps_t[:], bb_t[:, nsl])
                    s = spool.tile([P, 2], f32, name="s")
                    nc.vector.reduce_sum(
                        s[:],
                        y1[:].rearrange("p (b f) -> p b f", f=256),
                        axis=mybir.AxisListType.X,
                    )
                    nm = spool.tile([P, 2], f32, name="nm")
                    nc.vector.tensor_scalar_mul(nm[:], s[:], -1.0 / OUT_F)
                    g = gpool.tile([P, 512], f32, name="g")
                    for h in range(2):
                        nc.scalar.activation(
                            g[:, h * 256 : (h + 1) * 256],
                            y1[:, h * 256 : (h + 1) * 256],
                            AF.Gelu_apprx_tanh,
                            bias=nm[:, h : h + 1],
                        )
                    nc.sync.dma_start(out_d.ap()[m * P : (m + 1) * P, nsl], g[:])
                    return
                for h in range(2):
                    hsl = slice(h * 256, (h + 1) * 256)
                    y1 = ypool.tile([P, 256], f32, name="y1h")
                    nc.vector.tensor_add(y1[:], ps_t[:, hsl], bb_t[:, n * 512 + h * 256 : n * 512 + (h + 1) * 256])
                    s = spool.tile([P, 1], f32, name="sh")
                    nc.vector.reduce_sum(
                        s[:],
                        y1[:].rearrange("p (b f) -> p b f", f=256),
                        axis=mybir.AxisListType.X,
                    )
                    nm = spool.tile([P, 1], f32, name="nmh")
                    nc.vector.tensor_scalar_mul(nm[:], s[:], -1.0 / OUT_F)
                    g = gpool.tile([P, 256], f32, name="gh")
                    nc.scalar.activation(
                        g[:], y1[:], AF.Gelu_apprx_tanh, bias=nm[:, 0:1]
                    )
                    # two 64KB out DMAs so the last HBM write receipt
                    # (~1us round trip) pipelines behind the first
                    for d in range(2):
                        c0 = n * 512 + h * 256 + d * 128
                        nc.sync.dma_start(
                            out_d.ap()[m * P : (m + 1) * P, c0 : c0 + 128],
                            g[:, d * 128 : (d + 1) * 128],
                        )

            KH = KO // 2  # 16 ko per x half-tile
            KQ = KO // 4  # 8 ko per warm-phase x quarter-tile
            WSLAB = 4     # ko per W slab DMA (1MB transfers)

            def load_x(m):
                """Two half-tiles per m (1MB DMAs, finer PE wake-up).
                Returns a pieces list [(ko_lo, tile), ...]."""
                xa = xpool.tile([P, KH, P], bf16, name="xta")
                nc.sync.dma_start(xa[:], xs_d.ap()[:, m, 0:KH])
                xb = xpool.tile([P, KH, P], bf16, name="xtb")
                nc.sync.dma_start(xb[:], xs_d.ap()[:, m, KH:KO])
                return [(0, xa), (KH, xb)]

            def x_slice(pieces, ko):
                for lo, t in reversed(pieces):
                    if ko >= lo:
                        return t[:, ko - lo]
                raise AssertionError(ko)

            def body():
                G = warm_group
                assert G == 4, "phase-0 DMA wave is hardcoded for warm_group=4"
                # -- PE warm-up: the HAM clock-gate needs ~3.4us of sustained
                # PE activity to lift the cold 1.2GHz throttle, and the real
                # MM stream is DMA-paced for its first ~2us. Throwaway MMs on
                # a zeroed scratch tile keep the PE busy through both, so the
                # real stream runs at 2.4GHz from its first instruction.
                wsc = xpool.tile([P, 512], bf16, name="wsc")
                nc.vector.memset(wsc[:], 0.0)
                wps = psum.tile([P, 512], f32, name="ps")
                for _ in range(WARMUP_MMS):
                    nc.tensor.matmul(
                        wps[:], wsc[:, 0:128], wsc[:], start=True, stop=True
                    )
                # -- phase 0: DMA wave paced to the phase-1 ko-wavefront
                # consumption rate. The warm group's x streams as 256KB
                # quarter-tiles interleaved with the W slabs so neither
                # stream starves the other on bandwidth-poor cores: the
                # wavefront needs 0.25MB of W per 1.7us ko-wave plus one
                # x quarter per m every 8 waves.
                xts = {}
                slab_kos = [1, 1, 2] + [WSLAB] * ((KO - 4) // WSLAB)  # ko per slab
                assert sum(slab_kos) == KO
                slab_start = [sum(slab_kos[:i]) for i in range(len(slab_kos))]
                ko_to_slab = {}
                for i, (st, ln) in enumerate(zip(slab_start, slab_kos)):
                    for j in range(ln):
                        ko_to_slab[st + j] = (i, j)
                wts = [None] * len(slab_kos)

                def load_slab(sl):
                    st, ln = slab_start[sl], slab_kos[sl]
                    wt = wpool.tile([P, ln, NB], bf16, name=f"wt{sl}")
                    nc.sync.dma_start(wt[:], ws_d.ap()[:, st : st + ln])
                    wts[sl] = wt

                for m in range(G):
                    xts[m] = []

                def load_xq(m, q):
                    # q=0 feeds the first ko-waves: critical, Sync ring.
                    # Later quarters ride the second HWDGE ring (Scalar) so
                    # they never queue ahead of W slabs the PE needs sooner.
                    eng = nc.sync if q == 0 else nc.scalar
                    t = xpool.tile([P, KQ, P], bf16, name=f"xq{q}")
                    eng.dma_start(t[:], xs_d.ap()[:, m, q * KQ : (q + 1) * KQ])
                    xts[m].append((q * KQ, t))

                load_slab(0)
                load_xq(0, 0)
                load_xq(1, 0)
                load_slab(1)
                load_xq(2, 0)
                load_xq(3, 0)
                load_slab(2)
                load_slab(3)
                load_slab(4)
                load_xq(0, 1)
                load_xq(1, 1)
                load_xq(2, 1)
                load_xq(3, 1)
                load_slab(5)
                load_xq(0, 2)
                load_xq(1, 2)
                load_xq(2, 2)
                load_xq(3, 2)
                load_slab(6)
                load_xq(0, 3)
                load_xq(1, 3)
                load_xq(2, 3)
                load_xq(3, 3)
                for sl in range(7, len(slab_kos)):
                    load_slab(sl)

                bb_t = wpool.tile([P, NB], f32, name="bb")
                nc.scalar.dma_start(bb_t[:], bb_d.ap())

                def wt_slice(ko, n):
                    sl, j = ko_to_slab[ko]
                    return wts[sl][:, j, n * 512 : (n + 1) * 512]

                # -- phase 1: warm group, k-synchronous with W arrival
                if G:
                    ps1 = {
                        (m, n): psum.tile([P, 512], f32, name="ps")
                        for m in range(G)
                        for n in range(N_TILES)
                    }
                    # diagonal wavefront: ko-blocks aligned to W slabs, m
                    # inner — each DMA arrival unlocks one block
                    for st, ln in zip(slab_start, slab_kos):
                        for m in range(G):
                            for ko in range(st, st + ln):
                                for n in range(N_TILES):
                                    nc.tensor.matmul(
                                        ps1[m, n][:],
                                        x_slice(xts[m], ko),
                                        wt_slice(ko, n),
                                        start=(ko == 0),
                                        stop=(ko == KO - 1),
                                    )
                    # prefetch next x chunk (reuses slots freed at phase-1 end)
                    if G < M_TILES:
                        xts[G] = load_x(G)
                    for m in range(G):
                        del xts[m]
                        for n in range(N_TILES):
                            epilogue(m, n, ps1[m, n], bb_t)

                # -- phase 2: remaining m-tiles, k-inner per tile
                for m in range(G, M_TILES):
                    if m + 1 < M_TILES and (m + 1) not in xts:
                        xts[m + 1] = load_x(m + 1)
                    xt = xts.pop(m)
                    ps = [
                        psum.tile([P, 512], f32, name="ps") for _ in range(N_TILES)
                    ]
                    # n-outer: ps[n] finishes its full ko sweep before ps[n+1]
                    # starts, so each epilogue hides under the next MM block
                    for n in range(N_TILES):
                        for ko in range(KO):
                            nc.tensor.matmul(
                                ps[n][:],
                                x_slice(xt, ko),
                                wt_slice(ko, n),
                                start=(ko == 0),
                                stop=(ko == KO - 1),
                            )
                        last = m == M_TILES - 1 and n == N_TILES - 1
                        epilogue(m, n, ps[n], bb_t, split=last)

            if loop_reps is None:
                body()
            else:
                # straight-line replication with all-engine barriers between
                # reps: timing diff (R_hi - R_lo) isolates one cold run
                for r in range(loop_reps):
                    if r:
                        tc.strict_bb_all_engine_barrier()
                    body()

    nc.compile()
    return nc


def _make_runner(nc):
    """Jitted 8-core shard_map runner for a compiled Bass module."""
    import jax
    from jax.experimental.shard_map import shard_map
    from jax.sharding import Mesh, PartitionSpec
    from concourse import mybir
    from concourse.bass2jax import (
        _bass_exec_p,
        install_neuronx_cc_hook,
        partition_id_tensor,
    )

    install_neuronx_cc_hook()

    partition_name = nc.partition_id_tensor.name if nc.partition_id_tensor else None
    in_names = []
    out_names = []
    out_avals = []
    for alloc in nc.m.functions[0].allocations:
        if not isinstance(alloc, mybir.MemoryLocationSet):
            continue
        name = alloc.memorylocations[0].name
        if alloc.kind == "ExternalInput":
            if name != partition_name:
                in_names.append(name)
        elif alloc.kind == "ExternalOutput":
            out_names.append(name)
            out_avals.append(
                jax.core.ShapedArray(
                    tuple(alloc.tensor_shape), mybir.dt.np(alloc.dtype)
                )
            )
    n_params = len(in_names)
    all_names = in_names + out_names
    if partition_name is not None:
        all_names = all_names + [partition_name]

    def _body(*args):
        operands = list(args)
        if partition_name is not None:
            operands.append(partition_id_tensor())
        outs = _bass_exec_p.bind(
            *operands,
            out_avals=tuple(out_avals),
            in_names=tuple(all_names),
            out_names=tuple(out_names),
            lowering_input_output_aliases=(),
            sim_require_finite=True,
            sim_require_nnan=True,
            nc=nc,
        )
        return tuple(outs)

    devices = jax.devices()[:N_CORES]
    mesh = Mesh(np.asarray(devices), ("core",))
    n_outs = len(out_names)
    fn = jax.jit(
        shard_map(
            _body,
            mesh=mesh,
            in_specs=(PartitionSpec("core"),) * (n_params + n_outs),
            out_specs=(PartitionSpec("core"),) * n_outs,
            check_rep=False,
        ),
        keep_unused=True,
    )
    return fn, tuple(in_names), out_avals


def _get_runner():
    if "runner" not in _STATE:
        _STATE["runner"] = _make_runner(_build_bass())
    return _STATE["runner"]


def _prepare_inputs(x, weight, bias):
    """Round + shard + swizzle. Returns dict name -> concatenated (8*dim0)
    numpy array."""
    xr = _to_bf16(x)
    wr = _to_bf16(weight)
    bias = np.ascontiguousarray(bias, dtype=np.float32)

    xs_l, ws_l, bb_l = [], [], []
    for c in range(N_CORES):
        bi, oj = divmod(c, P_O)
        xc = xr[bi * MB : (bi + 1) * MB, :]
        # [p, m, ko, b] = xc[m*128+b, ko*128+p]
        xs_l.append(
            np.ascontiguousarray(xc.reshape(M_TILES, P, KO, P).transpose(3, 0, 2, 1))
        )
        wc = wr[oj * NB : (oj + 1) * NB, :]
        # [p, ko, n] = wc[n, ko*128+p]
        ws_l.append(np.ascontiguousarray(wc.reshape(NB, KO, P).transpose(2, 1, 0)))
        bb_l.append(
            np.ascontiguousarray(np.broadcast_to(bias[oj * NB : (oj + 1) * NB], (P, NB)))
        )
    return {
        "xs": np.concatenate(xs_l, axis=0),
        "ws": np.concatenate(ws_l, axis=0),
        "bb": np.concatenate(bb_l, axis=0),
    }


def _assemble(out_concat: np.ndarray) -> np.ndarray:
    """[8*2048, 1024] per-core stack -> full [4096, 4096]."""
    y = np.empty((B, OUT_F), np.float32)
    per = out_concat.reshape(N_CORES, MB, NB)
    for c in range(N_CORES):
        bi, oj = divmod(c, P_O)
        y[bi * MB : (bi + 1) * MB, oj * NB : (oj + 1) * NB] = per[c]
    return y


def kernel(x: np.ndarray, weight: np.ndarray, bias: np.ndarray) -> np.ndarray:
    fn, param_names, out_avals = _get_runner()
    ins = _prepare_inputs(np.asarray(x), np.asarray(weight), np.asarray(bias))
    args = [ins[n] for n in param_names]
    zeros = [
        np.zeros((N_CORES * a.shape[0], *a.shape[1:]), a.dtype) for a in out_avals
    ]
    outs = fn(*args, *zeros)
    return _assemble(np.asarray(outs[0]))



# revision 23
# speedup vs baseline: 1.0525x; 1.0525x over previous
"""Trainium2 Bass kernel for nn_ModelNew_3556232921881 (dense_mlp).

Computes, for x[4096,4096], weight[4096,4096], bias[4096]:
    y = x @ weight.T + bias
    per-256-column-block mean subtraction (divided by out_features)
    tanh-approx GELU with clamped tanh

Sharding: 2 batch shards x 4 out-feature shards across 8 NeuronCores.
Per core: M=2048, N=1024, K=4096 GEMM (bf16 full-rate matmul, fp32 PSUM
accumulate) with a fused epilogue (bias add -> block reduce ->
Gelu_apprx_tanh with the negated block mean as per-partition activation
bias).

bf16 operands (vs the earlier fp32r version) because the fp32r
LDWEIGHTS is a 2-cycle-per-column load (~227ns) that cannot hide under
the 213ns N=512 matmul, capping the steady-state MM issue rate at
~272ns. bf16 LDWEIGHTS (~53-107ns, FWL-eligible) hides completely ->
~213ns/MM, and the W/x DMA footprint halves (W shard 8MB SBUF-resident,
1MB x half-tiles). Accumulation is fp32 in PSUM; measured rel err vs
the fp32 reference is ~2e-3 (gate 2e-2).

Host side pre-rounds x/weight to bf16 (RNE) and swizzles them into the
exact SBUF layouts so the device does zero transposes or dtype
conversions. The first 4 m-tiles run k-synchronously with the W DMA
stream so the PE never waits for the W preload.
"""

import numpy as np
from contextlib import ExitStack

B, IN_F, OUT_F = 4096, 4096, 4096
P_B, P_O = 2, 4          # batch shards x out-feature shards
MB = B // P_B            # 2048 rows per core
NB = OUT_F // P_O        # 1024 out cols per core
K = IN_F
P = 128
M_TILES = MB // P        # 16
KO = K // P              # 32
N_TILES = NB // 512      # 2
N_CORES = 8
WARM_G = 4               # m-tiles processed k-synchronously with W stream
WARMUP_MMS = 12          # throwaway MMs to lift the HAM clock-gate early

_STATE: dict = {}


def _to_bf16(a: np.ndarray) -> np.ndarray:
    """fp32 -> bf16 with round-to-nearest-even."""
    import ml_dtypes

    return np.ascontiguousarray(a, dtype=np.float32).astype(ml_dtypes.bfloat16)


def _build_bass(loop_reps=None, warm_group=WARM_G):
    import concourse.bass as bass  # noqa: F401
    import concourse.tile as tile
    from concourse import bacc, mybir

    f32 = mybir.dt.float32
    bf16 = mybir.dt.bfloat16
    AF = mybir.ActivationFunctionType

    nc = bacc.Bacc("TRN2", target_bir_lowering=False, debug=False)

    # element [p, m, ko, b] = xr[m*128+b, ko*128+p]  (per-core batch shard)
    xs_d = nc.dram_tensor("xs", [P, M_TILES, KO, P], bf16, kind="ExternalInput")
    # element [p, ko, n] = w[n, ko*128+p]            (per-core outf shard)
    ws_d = nc.dram_tensor("ws", [P, KO, NB], bf16, kind="ExternalInput")
    bb_d = nc.dram_tensor("bb", [P, NB], f32, kind="ExternalInput")
    out_d = nc.dram_tensor("out", [MB, NB], f32, kind="ExternalOutput")

    with tile.TileContext(nc) as tc:
        with ExitStack() as ctx:
            wpool = ctx.enter_context(tc.tile_pool(name="w", bufs=1))
            xpool = ctx.enter_context(tc.tile_pool(name="x", bufs=max(warm_group, 2)))
            ypool = ctx.enter_context(tc.tile_pool(name="y", bufs=2))
            gpool = ctx.enter_context(tc.tile_pool(name="g", bufs=2))
            spool = ctx.enter_context(tc.tile_pool(name="s", bufs=3))
            psum = ctx.enter_context(tc.tile_pool(name="ps", bufs=8, space="PSUM"))

            def epilogue(m, n, ps_t, bb_t):
                """Bias add -> per-256-block sum -> Gelu(bias=-mean)."""
                nsl = slice(n * 512, (n + 1) * 512)
                y1 = ypool.tile([P, 512], f32, name="y1")
                nc.vector.tensor_add(y1[:], ps_t[:], bb_t[:, nsl])
                s = spool.tile([P, 2], f32, name="s")
                nc.vector.reduce_sum(
                    s[:],
                    y1[:].rearrange("p (b f) -> p b f", f=256),
                    axis=mybir.AxisListType.X,
                )
                nm = spool.tile([P, 2], f32, name="nm")
                nc.vector.tensor_scalar_mul(nm[:], s[:], -1.0 / OUT_F)
                g = gpool.tile([P, 512], f32, name="g")
                for h in range(2):
                    nc.scalar.activation(
                        g[:, h * 256 : (h + 1) * 256],
                        y1[:, h * 256 : (h + 1) * 256],
                        AF.Gelu_apprx_tanh,
                        bias=nm[:, h : h + 1],
                    )
                nc.sync.dma_start(out_d.ap()[m * P : (m + 1) * P, nsl], g[:])

            def epilogue256(m, c0, ps_ap, bb_t, split_dma=False):
                """One 256-col block chain (used on the kernel tail)."""
                y1 = ypool.tile([P, 256], f32, name="y1h")
                nc.vector.tensor_add(y1[:], ps_ap, bb_t[:, c0 : c0 + 256])
                s = spool.tile([P, 1], f32, name="sh")
                nc.vector.reduce_sum(
                    s[:],
                    y1[:].rearrange("p (b f) -> p b f", f=256),
                    axis=mybir.AxisListType.X,
                )
                nm = spool.tile([P, 1], f32, name="nmh")
                nc.vector.tensor_scalar_mul(nm[:], s[:], -1.0 / OUT_F)
                g = gpool.tile([P, 256], f32, name="gh")
                nc.scalar.activation(g[:], y1[:], AF.Gelu_apprx_tanh, bias=nm[:, 0:1])
                if not split_dma:
                    nc.sync.dma_start(
                        out_d.ap()[m * P : (m + 1) * P, c0 : c0 + 256], g[:]
                    )
                    return
                # two 64KB out DMAs so the last HBM write receipt (~1us
                # round trip) pipelines behind the first
                for dd in range(2):
                    cc = c0 + dd * 128
                    nc.sync.dma_start(
                        out_d.ap()[m * P : (m + 1) * P, cc : cc + 128],
                        g[:, dd * 128 : (dd + 1) * 128],
                    )

            KH = KO // 2  # 16 ko per x half-tile
            KQ = KO // 4  # 8 ko per warm-phase x quarter-tile
            WSLAB = 4     # ko per W slab DMA (1MB transfers)

            def load_x(m):
                """Two half-tiles per m (1MB DMAs, finer PE wake-up).
                Returns a pieces list [(ko_lo, tile), ...]."""
                xa = xpool.tile([P, KH, P], bf16, name="xta")
                nc.sync.dma_start(xa[:], xs_d.ap()[:, m, 0:KH])
                xb = xpool.tile([P, KH, P], bf16, name="xtb")
                nc.sync.dma_start(xb[:], xs_d.ap()[:, m, KH:KO])
                return [(0, xa), (KH, xb)]

            def x_slice(pieces, ko):
                for lo, t in reversed(pieces):
                    if ko >= lo:
                        return t[:, ko - lo]
                raise AssertionError(ko)

            def body():
                G = warm_group
                assert G == 4, "phase-0 DMA wave is hardcoded for warm_group=4"
                # -- PE warm-up: the HAM clock-gate needs ~3.4us of sustained
                # PE activity to lift the cold 1.2GHz throttle, and the real
                # MM stream is DMA-paced for its first ~2us. Throwaway MMs on
                # a zeroed scratch tile keep the PE busy through both, so the
                # real stream runs at 2.4GHz from its first instruction.
                wsc = xpool.tile([P, 512], bf16, name="wsc")
                nc.vector.memset(wsc[:], 0.0)
                wps = psum.tile([P, 512], f32, name="ps")
                for _ in range(WARMUP_MMS):
                    nc.tensor.matmul(
                        wps[:], wsc[:, 0:128], wsc[:], start=True, stop=True
                    )
                # -- phase 0: DMA wave paced to the phase-1 ko-wavefront
                # consumption rate. The warm group's x streams as 256KB
                # quarter-tiles interleaved with the W slabs so neither
                # stream starves the other on bandwidth-poor cores: the
                # wavefront needs 0.25MB of W per 1.7us ko-wave plus one
                # x quarter per m every 8 waves.
                xts = {}
                slab_kos = [1, 1, 2] + [WSLAB] * ((KO - 4) // WSLAB)  # ko per slab
                assert sum(slab_kos) == KO
                slab_start = [sum(slab_kos[:i]) for i in range(len(slab_kos))]
                ko_to_slab = {}
                for i, (st, ln) in enumerate(zip(slab_start, slab_kos)):
                    for j in range(ln):
                        ko_to_slab[st + j] = (i, j)
                wts = [None] * len(slab_kos)

                def load_slab(sl):
                    st, ln = slab_start[sl], slab_kos[sl]
                    wt = wpool.tile([P, ln, NB], bf16, name=f"wt{sl}")
                    nc.sync.dma_start(wt[:], ws_d.ap()[:, st : st + ln])
                    wts[sl] = wt

                for m in range(G):
                    xts[m] = []

                def load_xq(m, q):
                    t = xpool.tile([P, KQ, P], bf16, name=f"xq{q}")
                    nc.sync.dma_start(t[:], xs_d.ap()[:, m, q * KQ : (q + 1) * KQ])
                    xts[m].append((q * KQ, t))

                load_slab(0)
                load_xq(0, 0)
                load_xq(1, 0)
                load_slab(1)
                load_xq(2, 0)
                load_xq(3, 0)
                load_slab(2)
                load_slab(3)
                load_slab(4)
                load_xq(0, 1)
                load_xq(1, 1)
                load_xq(2, 1)
                load_xq(3, 1)
                load_slab(5)
                load_xq(0, 2)
                load_xq(1, 2)
                load_xq(2, 2)
                load_xq(3, 2)
                load_slab(6)
                load_xq(0, 3)
                load_xq(1, 3)
                load_xq(2, 3)
                load_xq(3, 3)
                for sl in range(7, len(slab_kos)):
                    load_slab(sl)

                bb_t = wpool.tile([P, NB], f32, name="bb")
                nc.sync.dma_start(bb_t[:], bb_d.ap())

                def wt_slice(ko, n):
                    sl, j = ko_to_slab[ko]
                    return wts[sl][:, j, n * 512 : (n + 1) * 512]

                # -- phase 1: warm group, k-synchronous with W arrival
                if G:
                    ps1 = {
                        (m, n): psum.tile([P, 512], f32, name="ps")
                        for m in range(G)
                        for n in range(N_TILES)
                    }
                    # diagonal wavefront: ko-blocks aligned to W slabs, m
                    # inner — each DMA arrival unlocks one block
                    for st, ln in zip(slab_start, slab_kos):
                        for m in range(G):
                            for ko in range(st, st + ln):
                                for n in range(N_TILES):
                                    nc.tensor.matmul(
                                        ps1[m, n][:],
                                        x_slice(xts[m], ko),
                                        wt_slice(ko, n),
                                        start=(ko == 0),
                                        stop=(ko == KO - 1),
                                    )
                    # prefetch next x chunk (reuses slots freed at phase-1 end)
                    if G < M_TILES:
                        xts[G] = load_x(G)
                    for m in range(G):
                        del xts[m]
                        for n in range(N_TILES):
                            epilogue(m, n, ps1[m, n], bb_t)

                # -- phase 2: remaining m-tiles, k-inner per tile
                for m in range(G, M_TILES):
                    if m + 1 < M_TILES and (m + 1) not in xts:
                        xts[m + 1] = load_x(m + 1)
                    xt = xts.pop(m)
                    last_m = m == M_TILES - 1
                    # n-outer: ps[n] finishes its full ko sweep before ps[n+1]
                    # starts, so each epilogue hides under the next MM block
                    for n in range(N_TILES - 1):
                        ps = psum.tile([P, 512], f32, name="ps")
                        for ko in range(KO):
                            nc.tensor.matmul(
                                ps[:],
                                x_slice(xt, ko),
                                wt_slice(ko, n),
                                start=(ko == 0),
                                stop=(ko == KO - 1),
                            )
                        epilogue(m, n, ps, bb_t)
                    n = N_TILES - 1
                    if not last_m:
                        ps = psum.tile([P, 512], f32, name="ps")
                        for ko in range(KO):
                            nc.tensor.matmul(
                                ps[:],
                                x_slice(xt, ko),
                                wt_slice(ko, n),
                                start=(ko == 0),
                                stop=(ko == KO - 1),
                            )
                        epilogue(m, n, ps, bb_t)
                    else:
                        # very last 512-col block: two 256-col ko-sweeps, so
                        # the first block's epilogue chain hides under the
                        # second block's matmuls and only one short chain
                        # remains after the final MM
                        for h in range(2):
                            c0 = n * 512 + h * 256
                            ph = psum.tile([P, 512], f32, name="ps")
                            for ko in range(KO):
                                nc.tensor.matmul(
                                    ph[:, 0:256],
                                    x_slice(xt, ko),
                                    wts[ko_to_slab[ko][0]][
                                        :, ko_to_slab[ko][1], c0 : c0 + 256
                                    ],
                                    start=(ko == 0),
                                    stop=(ko == KO - 1),
                                )
                            epilogue256(m, c0, ph[:, 0:256], bb_t, split_dma=(h == 1))

            if loop_reps is None:
                body()
            else:
                # straight-line replication with all-engine barriers between
                # reps: timing diff (R_hi - R_lo) isolates one cold run
                for r in range(loop_reps):
                    if r:
                        tc.strict_bb_all_engine_barrier()
                    body()

    nc.compile()
    return nc


def _make_runner(nc):
    """Jitted 8-core shard_map runner for a compiled Bass module."""
    import jax
    from jax.experimental.shard_map import shard_map
    from jax.sharding import Mesh, PartitionSpec
    from concourse import mybir
    from concourse.bass2jax import (
        _bass_exec_p,
        install_neuronx_cc_hook,
        partition_id_tensor,
    )

    install_neuronx_cc_hook()

    partition_name = nc.partition_id_tensor.name if nc.partition_id_tensor else None
    in_names = []
    out_names = []
    out_avals = []
    for alloc in nc.m.functions[0].allocations:
        if not isinstance(alloc, mybir.MemoryLocationSet):
            continue
        name = alloc.memorylocations[0].name
        if alloc.kind == "ExternalInput":
            if name != partition_name:
                in_names.append(name)
        elif alloc.kind == "ExternalOutput":
            out_names.append(name)
            out_avals.append(
                jax.core.ShapedArray(
                    tuple(alloc.tensor_shape), mybir.dt.np(alloc.dtype)
                )
            )
    n_params = len(in_names)
    all_names = in_names + out_names
    if partition_name is not None:
        all_names = all_names + [partition_name]

    def _body(*args):
        operands = list(args)
        if partition_name is not None:
            operands.append(partition_id_tensor())
        outs = _bass_exec_p.bind(
            *operands,
            out_avals=tuple(out_avals),
            in_names=tuple(all_names),
            out_names=tuple(out_names),
            lowering_input_output_aliases=(),
            sim_require_finite=True,
            sim_require_nnan=True,
            nc=nc,
        )
        return tuple(outs)

    devices = jax.devices()[:N_CORES]
    mesh = Mesh(np.asarray(devices), ("core",))
    n_outs = len(out_names)
    fn = jax.jit(
        shard_map(
            _body,
            mesh=mesh,
            in_specs=(PartitionSpec("core"),) * (n_params + n_outs),
            out_specs=(PartitionSpec("core"),) * n_outs,
            check_rep=False,
        ),
        keep_unused=True,
    )
    return fn, tuple(in_names), out_avals


def _get_runner():
    if "runner" not in _STATE:
        _STATE["runner"] = _make_runner(_build_bass())
    return _STATE["runner"]


def _prepare_inputs(x, weight, bias):
    """Round + shard + swizzle. Returns dict name -> concatenated (8*dim0)
    numpy array."""
    xr = _to_bf16(x)
    wr = _to_bf16(weight)
    bias = np.ascontiguousarray(bias, dtype=np.float32)

    xs_l, ws_l, bb_l = [], [], []
    for c in range(N_CORES):
        bi, oj = divmod(c, P_O)
        xc = xr[bi * MB : (bi + 1) * MB, :]
        # [p, m, ko, b] = xc[m*128+b, ko*128+p]
        xs_l.append(
            np.ascontiguousarray(xc.reshape(M_TILES, P, KO, P).transpose(3, 0, 2, 1))
        )
        wc = wr[oj * NB : (oj + 1) * NB, :]
        # [p, ko, n] = wc[n, ko*128+p]
        ws_l.append(np.ascontiguousarray(wc.reshape(NB, KO, P).transpose(2, 1, 0)))
        bb_l.append(
            np.ascontiguousarray(np.broadcast_to(bias[oj * NB : (oj + 1) * NB], (P, NB)))
        )
    return {
        "xs": np.concatenate(xs_l, axis=0),
        "ws": np.concatenate(ws_l, axis=0),
        "bb": np.concatenate(bb_l, axis=0),
    }


def _assemble(out_concat: np.ndarray) -> np.ndarray:
    """[8*2048, 1024] per-core stack -> full [4096, 4096]."""
    y = np.empty((B, OUT_F), np.float32)
    per = out_concat.reshape(N_CORES, MB, NB)
    for c in range(N_CORES):
        bi, oj = divmod(c, P_O)
        y[bi * MB : (bi + 1) * MB, oj * NB : (oj + 1) * NB] = per[c]
    return y


def kernel(x: np.ndarray, weight: np.ndarray, bias: np.ndarray) -> np.ndarray:
    fn, param_names, out_avals = _get_runner()
    ins = _prepare_inputs(np.asarray(x), np.asarray(weight), np.asarray(bias))
    args = [ins[n] for n in param_names]
    zeros = [
        np.zeros((N_CORES * a.shape[0], *a.shape[1:]), a.dtype) for a in out_avals
    ]
    outs = fn(*args, *zeros)
    return _assemble(np.asarray(outs[0]))

